# revision 28
# baseline (speedup 1.0000x reference)
"""Trainium2 Bass kernel for nn_Attention_46858093199829.

Math note (why x and b are never read on-device):
    score[b,i,j] = x[b,i] @ wx + key[j] @ wk + b0
The x-dependent term and the bias are constant in j, so they cancel in
softmax over j:
    softmax_j(score[b,i,:]) = softmax(key @ wk)          (same for every b,i)
    out[b,i,:]              = softmax(key @ wk) @ value  (a single 3-vector)

Sharding: data-parallel over batch B=32 -> 4 batches per core. key/value/W
are replicated; every core computes the (identical) 3-vector and writes its
own (4,512,3) output shard.

Implementation: raw Bacc (no TileContext, no nc.Block) with manual
semaphores. Tile's scheduling scaffolding (event-sem vector clocks,
per-engine drains, entry/exit barriers) costs far more than this kernel's
~3us of real work, so the program is written out by hand and the framework's
hardcoded const-AP memsets and all-engine barriers are stripped from the
BIR before compilation.

Engine plan (32-partition x 64-row layout -> few DMA descriptors):
  ACT    key/value loads (HWDGE), exp, final store
  GPSIMD tiny W-broadcast load (SWDGE), overlapping key's round trip
  DVE    sk = key@wk (tensor_scalar + 2 fused scalar_tensor_tensor MACs),
         e*value partials via one 0-step-broadcast multiply + two reduces,
         final v* = num/Z divide-and-broadcast straight out of PSUM
  PE     ones(32,32).T @ pcat(32,4): cross-partition reduce AND broadcast
         of the 4 column sums in a single matmul
"""

import sys
import types

import numpy as np

import concourse.bacc as bacc
import concourse.bass as bass
from concourse import bass_isa, mybir
from concourse.bass_utils import run_bass_kernel_spmd


def _install_ntff_shim():
    """bass_utils' trace path imports antenv.axon_hooks, which some images
    lack. Provide it, backed by trn_boot's ctypes NTFF hook when available;
    returning None makes bass_utils skip tracing instead of crashing."""
    if "antenv.axon_hooks" in sys.modules:
        return
    try:
        import antenv

        m = types.ModuleType("antenv.axon_hooks")
        _state = {"hook": None, "tried": False}

        def set_axon_ntff_profile_hook(h):
            _state["hook"] = h
            _state["tried"] = True

        def get_axon_ntff_profile_hook():
            if not _state["tried"]:
                _state["tried"] = True
                try:
                    from trn_agent_boot.trn_boot import _ntff_profile_via_ctypes

                    _state["hook"] = _ntff_profile_via_ctypes(
                        "/opt/axon/libaxon_pjrt.so"
                    )
                except Exception:
                    _state["hook"] = None
            return _state["hook"]

        m.set_axon_ntff_profile_hook = set_axon_ntff_profile_hook
        m.get_axon_ntff_profile_hook = get_axon_ntff_profile_hook
        sys.modules["antenv.axon_hooks"] = m
        antenv.axon_hooks = m
    except Exception:
        pass


_install_ntff_shim()

N_CORES = 8
B, S1, S2, D = 32, 512, 2048, 3
ROWS_PER_CORE = (B // N_CORES) * S1  # 2048 rows of (3,) per core

# Exposed for the test harness: the BassKernelResults of the last run
# (carries exec_time_ns when BASS_TRACE=1).
last_result = None

_nc_cache = None


def _strip_framework_overhead(nc):
    """Remove Bass' hardcoded framework scaffolding: the const-AP memsets
    (nothing here reads them) and the entry/exit all-engine barriers
    (Drain + barrier_* EventSemaphores). All ordering in this kernel is
    carried by its own semaphores; the runtime resets semaphores per
    execution, so no cross-engine barrier is needed at entry or exit."""

    def is_barrier_sync(si):
        if si is None:
            return False
        refs = [w.ant_name for w in (si.on_wait or [])] + [
            u.ant_name for u in (si.on_update or [])
        ]
        return bool(refs) and all(r.startswith("barrier_") for r in refs)

    for f in nc.m.functions:
        for bb in f.blocks:
            kept = []
            for inst in bb.instructions:
                t = type(inst).__name__
                if t == "InstDrain":
                    continue
                if (
                    t == "InstMemset"
                    and inst.outs
                    and getattr(inst.outs[0], "memref", "").startswith("const-")
                ):
                    continue
                if t == "InstEventSemaphore" and is_barrier_sync(inst.sync_info):
                    continue
                kept.append(inst)
            if len(kept) != len(bb.instructions):
                bb.instructions[:] = kept


def _strip_unused_engines(nc, engines):
    """Remove every instruction on the given (unused) engines — only Bass'
    boot code (TPBBaseLd + RegisterMoves) lives there. An engine with an
    empty program gets no walrus-injected epilogue sweep; the trace then
    ends with the slowest *used* engine's sweep (PE's is the slowest)."""
    for f in nc.m.functions:
        for bb in f.blocks:
            kept = [
                inst
                for inst in bb.instructions
                if getattr(inst, "engine", None) not in engines
            ]
            if len(kept) != len(bb.instructions):
                bb.instructions[:] = kept


def _build():
    nc = bacc.Bacc(target_bir_lowering=False, debug=False)
    f32 = mybir.dt.float32
    key_t = nc.dram_tensor("key", [S2, D], f32, kind="ExternalInput")
    val_t = nc.dram_tensor("value", [S2, D], f32, kind="ExternalInput")
    w_t = nc.dram_tensor("W", [1, 6], f32, kind="ExternalInput")
    out_t = nc.dram_tensor("out", [ROWS_PER_CORE, D], f32, kind="ExternalOutput")

    from contextlib import ExitStack

    # 32 partitions x 64 rows: 4x fewer DMA descriptors per transfer than a
    # 128-partition layout; DVE/ACT op time is overhead-dominated either way.
    Q, QF = 32, 64  # 2048 = 32 * 64

    with ExitStack() as ctx:
        ec = ctx.enter_context
        kv = ec(nc.sbuf_tensor("kv", [Q, QF, D], f32))
        vv = ec(nc.sbuf_tensor("vv", [Q, QF, D], f32))
        wb = ec(nc.sbuf_tensor("wb", [Q, 6], f32))
        t0 = ec(nc.sbuf_tensor("t0", [Q, QF], f32))
        t1 = ec(nc.sbuf_tensor("t1", [Q, QF], f32))
        sk = ec(nc.sbuf_tensor("sk", [Q, QF], f32))
        e = ec(nc.sbuf_tensor("e", [Q, QF], f32))
        bz = ec(nc.sbuf_tensor("bz", [Q, 1], f32))
        mall = ec(nc.sbuf_tensor("mall", [Q, QF, D], f32))
        pcat = ec(nc.sbuf_tensor("pcat", [Q, 4], f32))
        red4 = ec(nc.sbuf_tensor("red4", [Q, 4], f32))
        rz = ec(nc.sbuf_tensor("rz", [Q, 1], f32))
        ot = ec(nc.sbuf_tensor("ot", [Q, QF, D], f32))
        dW = ec(nc.semaphore("dW"))      # W broadcast load
        dK = ec(nc.semaphore("dK"))      # key load
        dV = ec(nc.semaphore("dV"))      # value load
        dO = ec(nc.semaphore("dO"))      # final store
        vp = ec(nc.semaphore("vp"))      # vector same-engine pipeline sem
        s_sk = ec(nc.semaphore("s_sk"))  # vector -> scalar: sk ready
        s_e = ec(nc.semaphore("s_e"))    # scalar -> vector: e ready
        s_pc = ec(nc.semaphore("s_pc"))  # vector -> gpsimd: pcat ready (x2)
        s_mm = ec(nc.semaphore("s_mm"))  # gpsimd -> vector: reduction done
        s_ot = ec(nc.semaphore("s_ot"))  # vector -> scalar: out tile ready

        # No nc.Block(): straight-line single-bb program, engines' streams
        # interleave by engine tag; ordering is purely semaphore-driven.
        # Only three engines carry work: ACT (HWDGE DMAs + exp), DVE, PE.

        # --- loads: key + value on the ACT queue (HWDGE), the tiny W
        # broadcast on the otherwise-idle gpsimd queue (SWDGE) so its round
        # trip fully overlaps key's
        nc.scalar.dma_start(
            out=kv[:, :, :], in_=key_t[:, :].rearrange("(p n) d -> p n d", p=Q)
        ).then_inc(dK, 16)
        nc.scalar.dma_start(
            out=vv[:, :, :], in_=val_t[:, :].rearrange("(p n) d -> p n d", p=Q)
        ).then_inc(dV, 16)
        nc.gpsimd.dma_start(out=wb[:, :], in_=w_t[:, :].to_broadcast([Q, 6])).then_inc(
            dW, 16
        )

        # --- vector: sk[p,n] = sum_d key[p,n,d] * wk[d]  (wk = W[0, 3:6]) ---
        nc.vector.memset(bz[:, :], 0.0)
        nc.vector.wait_ge(dK, 16)
        nc.vector.wait_ge(dW, 16)
        nc.vector.tensor_scalar_mul(t0[:, :], kv[:, :, 0], wb[:, 3:4]).then_inc(vp)
        nc.vector.wait_ge(vp, 1)
        nc.vector.scalar_tensor_tensor(
            t1[:, :], kv[:, :, 1], wb[:, 4:5], t0[:, :],
            op0=mybir.AluOpType.mult, op1=mybir.AluOpType.add,
        ).then_inc(vp)
        nc.vector.wait_ge(vp, 2)
        nc.vector.scalar_tensor_tensor(
            sk[:, :], kv[:, :, 2], wb[:, 5:6], t1[:, :],
            op0=mybir.AluOpType.mult, op1=mybir.AluOpType.add,
        ).then_inc(s_sk, 1)

        # --- scalar: e = exp(sk) ---
        nc.scalar.wait_ge(s_sk, 1)
        nc.scalar.activation(
            e[:, :], sk[:, :], mybir.ActivationFunctionType.Exp, bias=bz[:, 0:1]
        ).then_inc(s_e, 1)

        # --- vector: pcat[:,0:3] = per-partition sum_n e*value_d, [3] = sum e
        nc.vector.wait_ge(s_e, 1)
        nc.vector.wait_ge(dV, 16)
        e_b = bass.AP(tensor=e.ap().tensor, offset=0, ap=[[QF, Q], [1, QF], [0, D]])
        nc.vector.tensor_mul(mall[:, :, :], e_b, vv[:, :, :]).then_inc(vp)
        nc.vector.wait_ge(vp, 3)
        nc.vector.reduce_sum(
            pcat[:, 0:3],
            mall.ap().rearrange("p n d -> p d n"),
            axis=mybir.AxisListType.X,
        ).then_inc(s_pc, 1)
        nc.vector.reduce_sum(
            pcat[:, 3:4], e[:, :], axis=mybir.AxisListType.X
        ).then_inc(s_pc, 1)

        # --- gpsimd: all-reduce pcat across partitions (replaces a PE
        # matmul so the PE queue stays empty -> no PE epilogue sweep) ---
        nc.gpsimd.wait_ge(s_pc, 2)
        nc.gpsimd.partition_all_reduce(
            red4[:, :], pcat[:, :], channels=Q, reduce_op=bass_isa.ReduceOp.add
        ).then_inc(s_mm, 1)

        # --- vector: v* = num/Z, broadcast x64 into the out tile via a
        # 0-step middle dim on the input AP ---
        nc.vector.wait_ge(s_mm, 1)
        nc.vector.reciprocal(rz[:, :], red4[:, 3:4]).then_inc(vp)
        nc.vector.wait_ge(vp, 4)
        num_b = bass.AP(
            tensor=red4.ap().tensor, offset=0, ap=[[4, Q], [0, QF], [1, D]]
        )
        nc.vector.tensor_scalar_mul(ot[:, :, :], num_b, rz[:, 0:1]).then_inc(s_ot, 1)

        # --- scalar: store the full shard ---
        nc.scalar.wait_ge(s_ot, 1)
        nc.scalar.dma_start(
            out=out_t[:, :].rearrange("(p n) d -> p n d", p=Q), in_=ot[:, :, :]
        ).then_inc(dO, 16)
        nc.scalar.wait_ge(dO, 16)

    _strip_framework_overhead(nc)
    nc.compile()
    _strip_unused_engines(
        nc, (mybir.EngineType.PE, mybir.EngineType.SP)
    )
    return nc


def kernel(x, key, value, W, b):
    global last_result, _nc_cache
    key = np.ascontiguousarray(np.asarray(key, dtype=np.float32))
    value = np.ascontiguousarray(np.asarray(value, dtype=np.float32))
    W = np.ascontiguousarray(np.asarray(W, dtype=np.float32))
    if _nc_cache is None:
        _nc_cache = _build()
    in_maps = [
        {"key": key, "value": value, "W": W} for _ in range(N_CORES)
    ]
    res = run_bass_kernel_spmd(_nc_cache, in_maps, core_ids=list(range(N_CORES)))
    last_result = res
    out = np.concatenate([r["out"] for r in res.results], axis=0)
    return out.reshape(B, S1, D)


# revision 30
# speedup vs baseline: 1.3122x; 1.3122x over previous
"""Trainium2 Bass kernel for nn_Attention_46858093199829.

Math note (why x and b are never read on-device):
    score[b,i,j] = x[b,i] @ wx + key[j] @ wk + b0
The x-dependent term and the bias are constant in j, so they cancel in
softmax over j:
    softmax_j(score[b,i,:]) = softmax(key @ wk)          (same for every b,i)
    out[b,i,:]              = softmax(key @ wk) @ value  (a single 3-vector)

Sharding: data-parallel over batch B=32 -> 4 batches per core. key/value/W
are replicated; every core computes the (identical) 3-vector and writes its
own (4,512,3) output shard.

Implementation: raw Bacc (no TileContext, no nc.Block) with manual
semaphores. Tile's scheduling scaffolding (event-sem vector clocks,
per-engine drains, entry/exit barriers) costs far more than this kernel's
~3us of real work, so the program is written out by hand and the framework's
hardcoded const-AP memsets and all-engine barriers are stripped from the
BIR before compilation.

Engine plan (32-partition x 64-row layout -> few DMA descriptors):
  ACT    key/value loads (HWDGE), exp, final store
  GPSIMD tiny W-broadcast load (SWDGE), overlapping key's round trip
  DVE    sk = key@wk (tensor_scalar + 2 fused scalar_tensor_tensor MACs),
         e*value partials via one 0-step-broadcast multiply + two reduces,
         final v* = num/Z divide-and-broadcast straight out of PSUM
  PE     ones(32,32).T @ pcat(32,4): cross-partition reduce AND broadcast
         of the 4 column sums in a single matmul
"""

import sys
import types

import numpy as np

import concourse.bacc as bacc
import concourse.bass as bass
from concourse import mybir
from concourse.bass_utils import run_bass_kernel_spmd


def _install_ntff_shim():
    """bass_utils' trace path imports antenv.axon_hooks, which some images
    lack. Provide it, backed by trn_boot's ctypes NTFF hook when available;
    returning None makes bass_utils skip tracing instead of crashing."""
    if "antenv.axon_hooks" in sys.modules:
        return
    try:
        import antenv

        m = types.ModuleType("antenv.axon_hooks")
        _state = {"hook": None, "tried": False}

        def set_axon_ntff_profile_hook(h):
            _state["hook"] = h
            _state["tried"] = True

        def get_axon_ntff_profile_hook():
            if not _state["tried"]:
                _state["tried"] = True
                try:
                    from trn_agent_boot.trn_boot import _ntff_profile_via_ctypes

                    _state["hook"] = _ntff_profile_via_ctypes(
                        "/opt/axon/libaxon_pjrt.so"
                    )
                except Exception:
                    _state["hook"] = None
            return _state["hook"]

        m.set_axon_ntff_profile_hook = set_axon_ntff_profile_hook
        m.get_axon_ntff_profile_hook = get_axon_ntff_profile_hook
        sys.modules["antenv.axon_hooks"] = m
        antenv.axon_hooks = m
    except Exception:
        pass


_install_ntff_shim()

N_CORES = 8
B, S1, S2, D = 32, 512, 2048, 3
ROWS_PER_CORE = (B // N_CORES) * S1  # 2048 rows of (3,) per core

# Exposed for the test harness: the BassKernelResults of the last run
# (carries exec_time_ns when BASS_TRACE=1).
last_result = None

_nc_cache = None


def _strip_framework_overhead(nc):
    """Remove Bass' hardcoded framework scaffolding: the const-AP memsets
    (nothing here reads them) and the entry/exit all-engine barriers
    (Drain + barrier_* EventSemaphores). All ordering in this kernel is
    carried by its own semaphores; the runtime resets semaphores per
    execution, so no cross-engine barrier is needed at entry or exit."""

    def is_barrier_sync(si):
        if si is None:
            return False
        refs = [w.ant_name for w in (si.on_wait or [])] + [
            u.ant_name for u in (si.on_update or [])
        ]
        return bool(refs) and all(r.startswith("barrier_") for r in refs)

    for f in nc.m.functions:
        for bb in f.blocks:
            kept = []
            for inst in bb.instructions:
                t = type(inst).__name__
                if t == "InstDrain":
                    continue
                if (
                    t == "InstMemset"
                    and inst.outs
                    and getattr(inst.outs[0], "memref", "").startswith("const-")
                ):
                    continue
                if t == "InstEventSemaphore" and is_barrier_sync(inst.sync_info):
                    continue
                kept.append(inst)
            if len(kept) != len(bb.instructions):
                bb.instructions[:] = kept


def _build():
    nc = bacc.Bacc(target_bir_lowering=False, debug=False)
    f32 = mybir.dt.float32
    key_t = nc.dram_tensor("key", [S2, D], f32, kind="ExternalInput")
    val_t = nc.dram_tensor("value", [S2, D], f32, kind="ExternalInput")
    w_t = nc.dram_tensor("W", [1, 6], f32, kind="ExternalInput")
    out_t = nc.dram_tensor("out", [ROWS_PER_CORE, D], f32, kind="ExternalOutput")

    from contextlib import ExitStack

    # 32 partitions x 64 rows: 4x fewer DMA descriptors per transfer than a
    # 128-partition layout; DVE/ACT op time is overhead-dominated either way.
    Q, QF = 32, 64  # 2048 = 32 * 64

    with ExitStack() as ctx:
        ec = ctx.enter_context
        kv = ec(nc.sbuf_tensor("kv", [Q, QF, D], f32))
        vv = ec(nc.sbuf_tensor("vv", [Q, QF, D], f32))
        wb = ec(nc.sbuf_tensor("wb", [Q, 6], f32))
        t0 = ec(nc.sbuf_tensor("t0", [Q, QF], f32))
        t1 = ec(nc.sbuf_tensor("t1", [Q, QF], f32))
        sk = ec(nc.sbuf_tensor("sk", [Q, QF], f32))
        e = ec(nc.sbuf_tensor("e", [Q, QF], f32))
        bz = ec(nc.sbuf_tensor("bz", [Q, 1], f32))
        mall = ec(nc.sbuf_tensor("mall", [Q, QF, D], f32))
        pcat = ec(nc.sbuf_tensor("pcat", [Q, 4], f32))
        ones = ec(nc.sbuf_tensor("ones", [Q, Q], f32))
        rz = ec(nc.sbuf_tensor("rz", [Q, 1], f32))
        ot = ec(nc.sbuf_tensor("ot", [Q, QF, D], f32))
        redp = ec(nc.psum_tensor("redp", [Q, 4], f32))
        dW = ec(nc.semaphore("dW"))      # W broadcast load
        dK = ec(nc.semaphore("dK"))      # key load
        dV = ec(nc.semaphore("dV"))      # value load
        dO = ec(nc.semaphore("dO"))      # final store
        vp = ec(nc.semaphore("vp"))      # vector same-engine pipeline sem
        s_sk = ec(nc.semaphore("s_sk"))  # vector -> scalar: sk ready
        s_e = ec(nc.semaphore("s_e"))    # scalar -> vector: e ready
        s_pc = ec(nc.semaphore("s_pc"))  # vector -> tensor: pcat ready (x2)
        s_mm = ec(nc.semaphore("s_mm"))  # tensor -> vector: reduction done
        s_ot = ec(nc.semaphore("s_ot"))  # vector -> scalar: out tile ready

        # No nc.Block(): straight-line single-bb program, engines' streams
        # interleave by engine tag; ordering is purely semaphore-driven.
        # Only three engines carry work: ACT (HWDGE DMAs + exp), DVE, PE.

        # --- loads: key + value on the ACT queue (HWDGE), the tiny W
        # broadcast on the otherwise-idle gpsimd queue (SWDGE) so its round
        # trip fully overlaps key's
        nc.scalar.dma_start(
            out=kv[:, :, :], in_=key_t[:, :].rearrange("(p n) d -> p n d", p=Q)
        ).then_inc(dK, 16)
        nc.scalar.dma_start(
            out=vv[:, :, :], in_=val_t[:, :].rearrange("(p n) d -> p n d", p=Q)
        ).then_inc(dV, 16)
        nc.gpsimd.dma_start(out=wb[:, :], in_=w_t[:, :].to_broadcast([Q, 6])).then_inc(
            dW, 16
        )

        # --- vector: sk[p,n] = sum_d key[p,n,d] * wk[d]  (wk = W[0, 3:6]) ---
        nc.vector.memset(bz[:, :], 0.0)
        nc.vector.memset(ones[:, :], 1.0)
        nc.vector.wait_ge(dK, 16)
        nc.vector.wait_ge(dW, 16)
        nc.vector.tensor_scalar_mul(t0[:, :], kv[:, :, 0], wb[:, 3:4]).then_inc(vp)
        nc.vector.wait_ge(vp, 1)
        nc.vector.scalar_tensor_tensor(
            t1[:, :], kv[:, :, 1], wb[:, 4:5], t0[:, :],
            op0=mybir.AluOpType.mult, op1=mybir.AluOpType.add,
        ).then_inc(vp)
        nc.vector.wait_ge(vp, 2)
        nc.vector.scalar_tensor_tensor(
            sk[:, :], kv[:, :, 2], wb[:, 5:6], t1[:, :],
            op0=mybir.AluOpType.mult, op1=mybir.AluOpType.add,
        ).then_inc(s_sk, 1)

        # --- scalar: e = exp(sk) ---
        nc.scalar.wait_ge(s_sk, 1)
        nc.scalar.activation(
            e[:, :], sk[:, :], mybir.ActivationFunctionType.Exp, bias=bz[:, 0:1]
        ).then_inc(s_e, 1)

        # --- vector: pcat[:,0:3] = per-partition sum_n e*value_d, [3] = sum e
        nc.vector.wait_ge(s_e, 1)
        nc.vector.wait_ge(dV, 16)
        e_b = bass.AP(tensor=e.ap().tensor, offset=0, ap=[[QF, Q], [1, QF], [0, D]])
        nc.vector.tensor_mul(mall[:, :, :], e_b, vv[:, :, :]).then_inc(vp)
        nc.vector.wait_ge(vp, 3)
        nc.vector.reduce_sum(
            pcat[:, 0:3],
            mall.ap().rearrange("p n d -> p d n"),
            axis=mybir.AxisListType.X,
        ).then_inc(s_pc, 1)
        nc.vector.reduce_sum(
            pcat[:, 3:4], e[:, :], axis=mybir.AxisListType.X
        ).then_inc(s_pc, 1)

        # --- tensor: ones(32,32).T @ pcat(32,4) reduces across partitions
        # AND broadcasts the 4 column sums to all 32 partitions in one op ---
        nc.tensor.wait_ge(s_pc, 2)
        nc.tensor.matmul(
            redp.ap()[:, :], ones[:, :], pcat[:, :], start=True, stop=True
        ).then_inc(s_mm, 1)

        # --- vector: v* = num/Z straight out of PSUM, broadcast x64 into the
        # out tile via a 0-step middle dim on the input AP ---
        nc.vector.wait_ge(s_mm, 1)
        nc.vector.reciprocal(rz[:, :], redp.ap()[:, 3:4]).then_inc(vp)
        nc.vector.wait_ge(vp, 4)
        num_b = bass.AP(
            tensor=redp.ap().tensor, offset=0, ap=[[4, Q], [0, QF], [1, D]]
        )
        nc.vector.tensor_scalar_mul(ot[:, :, :], num_b, rz[:, 0:1]).then_inc(s_ot, 1)

        # --- scalar: store the full shard. No explicit dO wait: the
        # runtime-injected epilogue DRAIN on ACT covers the in-flight DMA,
        # so its round trip overlaps the (fixed) exit sweep ---
        nc.scalar.wait_ge(s_ot, 1)
        nc.scalar.dma_start(
            out=out_t[:, :].rearrange("(p n) d -> p n d", p=Q), in_=ot[:, :, :]
        ).then_inc(dO, 16)

    _strip_framework_overhead(nc)
    nc.compile()
    return nc


def kernel(x, key, value, W, b):
    global last_result, _nc_cache
    key = np.ascontiguousarray(np.asarray(key, dtype=np.float32))
    value = np.ascontiguousarray(np.asarray(value, dtype=np.float32))
    W = np.ascontiguousarray(np.asarray(W, dtype=np.float32))
    if _nc_cache is None:
        _nc_cache = _build()
    in_maps = [
        {"key": key, "value": value, "W": W} for _ in range(N_CORES)
    ]
    res = run_bass_kernel_spmd(_nc_cache, in_maps, core_ids=list(range(N_CORES)))
    last_result = res
    out = np.concatenate([r["out"] for r in res.results], axis=0)
    return out.reshape(B, S1, D)


# revision 33
# speedup vs baseline: 1.3543x; 1.0321x over previous
"""Trainium2 Bass kernel for nn_Attention_46858093199829.

Math note (why x and b are never read on-device):
    score[b,i,j] = x[b,i] @ wx + key[j] @ wk + b0
The x-dependent term and the bias are constant in j, so they cancel in
softmax over j:
    softmax_j(score[b,i,:]) = softmax(key @ wk)          (same for every b,i)
    out[b,i,:]              = softmax(key @ wk) @ value  (a single 3-vector)

Sharding: data-parallel over batch B=32 -> 4 batches per core. key/value/W
are replicated; every core computes the (identical) 3-vector and writes its
own (4,512,3) output shard.

Implementation: raw Bacc (no TileContext, no nc.Block) with manual
semaphores. Tile's scheduling scaffolding (event-sem vector clocks,
per-engine drains, entry/exit barriers) costs far more than this kernel's
~3us of real work, so the program is written out by hand and the framework's
hardcoded const-AP memsets and all-engine barriers are stripped from the
BIR before compilation.

Engine plan (32-partition x 64-row layout -> few DMA descriptors):
  ACT    key/value loads (HWDGE), exp, final store
  GPSIMD tiny W-broadcast load (SWDGE), overlapping key's round trip
  DVE    sk = key@wk (tensor_scalar + 2 fused scalar_tensor_tensor MACs),
         e*value partials via one 0-step-broadcast multiply + two reduces,
         final v* = num/Z divide-and-broadcast straight out of PSUM
  PE     ones(32,32).T @ pcat(32,4): cross-partition reduce AND broadcast
         of the 4 column sums in a single matmul
"""

import sys
import types

import numpy as np

import concourse.bacc as bacc
import concourse.bass as bass
from concourse import mybir
from concourse.bass_utils import run_bass_kernel_spmd


def _install_ntff_shim():
    """bass_utils' trace path imports antenv.axon_hooks, which some images
    lack. Provide it, backed by trn_boot's ctypes NTFF hook when available;
    returning None makes bass_utils skip tracing instead of crashing."""
    if "antenv.axon_hooks" in sys.modules:
        return
    try:
        import antenv

        m = types.ModuleType("antenv.axon_hooks")
        _state = {"hook": None, "tried": False}

        def set_axon_ntff_profile_hook(h):
            _state["hook"] = h
            _state["tried"] = True

        def get_axon_ntff_profile_hook():
            if not _state["tried"]:
                _state["tried"] = True
                try:
                    from trn_agent_boot.trn_boot import _ntff_profile_via_ctypes

                    _state["hook"] = _ntff_profile_via_ctypes(
                        "/opt/axon/libaxon_pjrt.so"
                    )
                except Exception:
                    _state["hook"] = None
            return _state["hook"]

        m.set_axon_ntff_profile_hook = set_axon_ntff_profile_hook
        m.get_axon_ntff_profile_hook = get_axon_ntff_profile_hook
        sys.modules["antenv.axon_hooks"] = m
        antenv.axon_hooks = m
    except Exception:
        pass


_install_ntff_shim()

N_CORES = 8
B, S1, S2, D = 32, 512, 2048, 3
ROWS_PER_CORE = (B // N_CORES) * S1  # 2048 rows of (3,) per core

# Exposed for the test harness: the BassKernelResults of the last run
# (carries exec_time_ns when BASS_TRACE=1).
last_result = None

_nc_cache = None


def _strip_framework_overhead(nc):
    """Remove Bass' hardcoded framework scaffolding: the const-AP memsets
    (nothing here reads them) and the entry/exit all-engine barriers
    (Drain + barrier_* EventSemaphores). All ordering in this kernel is
    carried by its own semaphores; the runtime resets semaphores per
    execution, so no cross-engine barrier is needed at entry or exit."""

    def is_barrier_sync(si):
        if si is None:
            return False
        refs = [w.ant_name for w in (si.on_wait or [])] + [
            u.ant_name for u in (si.on_update or [])
        ]
        return bool(refs) and all(r.startswith("barrier_") for r in refs)

    for f in nc.m.functions:
        for bb in f.blocks:
            kept = []
            for inst in bb.instructions:
                t = type(inst).__name__
                if t == "InstDrain":
                    continue
                if (
                    t == "InstMemset"
                    and inst.outs
                    and getattr(inst.outs[0], "memref", "").startswith("const-")
                ):
                    continue
                if t == "InstEventSemaphore" and is_barrier_sync(inst.sync_info):
                    continue
                kept.append(inst)
            if len(kept) != len(bb.instructions):
                bb.instructions[:] = kept


def _build():
    nc = bacc.Bacc(target_bir_lowering=False, debug=False)
    f32 = mybir.dt.float32
    key_t = nc.dram_tensor("key", [S2, D], f32, kind="ExternalInput")
    val_t = nc.dram_tensor("value", [S2, D], f32, kind="ExternalInput")
    w_t = nc.dram_tensor("W", [1, 6], f32, kind="ExternalInput")
    out_t = nc.dram_tensor("out", [ROWS_PER_CORE, D], f32, kind="ExternalOutput")

    from contextlib import ExitStack

    # 32 partitions x 64 rows: 4x fewer DMA descriptors per transfer than a
    # 128-partition layout; DVE/ACT op time is overhead-dominated either way.
    Q, QF = 32, 64  # 2048 = 32 * 64

    with ExitStack() as ctx:
        ec = ctx.enter_context
        kv = ec(nc.sbuf_tensor("kv", [Q, QF, D], f32))
        vv = ec(nc.sbuf_tensor("vv", [Q, QF, D], f32))
        wb = ec(nc.sbuf_tensor("wb", [Q, 6], f32))
        prod = ec(nc.sbuf_tensor("prod", [Q, QF, D], f32))
        sk = ec(nc.sbuf_tensor("sk", [Q, QF], f32))
        bz = ec(nc.sbuf_tensor("bz", [Q, 1], f32))
        # d-major (Q, 4, QF): rows 0-2 = e*value_d, row 3 = e itself, so one
        # reduce yields all four column partials at once
        mall4 = ec(nc.sbuf_tensor("mall4", [Q, 4, QF], f32))
        pcat = ec(nc.sbuf_tensor("pcat", [Q, 4], f32))
        ones = ec(nc.sbuf_tensor("ones", [Q, Q], f32))
        rz = ec(nc.sbuf_tensor("rz", [Q, 1], f32))
        ot = ec(nc.sbuf_tensor("ot", [Q, QF, D], f32))
        redp = ec(nc.psum_tensor("redp", [Q, 4], f32))
        dW = ec(nc.semaphore("dW"))      # W broadcast load
        dK = ec(nc.semaphore("dK"))      # key load
        dV = ec(nc.semaphore("dV"))      # value load
        dO = ec(nc.semaphore("dO"))      # final store
        vp = ec(nc.semaphore("vp"))      # vector same-engine pipeline sem
        s_sk = ec(nc.semaphore("s_sk"))  # vector -> scalar: sk ready
        s_e = ec(nc.semaphore("s_e"))    # scalar -> vector: e ready
        s_pc = ec(nc.semaphore("s_pc"))  # vector -> tensor: pcat ready (x2)
        s_mm = ec(nc.semaphore("s_mm"))  # tensor -> vector: reduction done
        s_ot = ec(nc.semaphore("s_ot"))  # vector -> scalar: out tile ready

        # No nc.Block(): straight-line single-bb program, engines' streams
        # interleave by engine tag; ordering is purely semaphore-driven.
        # Only three engines carry work: ACT (HWDGE DMAs + exp), DVE, PE.

        # --- loads: key + value on the ACT queue (HWDGE), the tiny W
        # broadcast on the otherwise-idle gpsimd queue (SWDGE) so its round
        # trip fully overlaps key's
        nc.scalar.dma_start(
            out=kv[:, :, :], in_=key_t[:, :].rearrange("(p n) d -> p n d", p=Q)
        ).then_inc(dK, 16)
        nc.scalar.dma_start(
            out=vv[:, :, :], in_=val_t[:, :].rearrange("(p n) d -> p n d", p=Q)
        ).then_inc(dV, 16)
        nc.gpsimd.dma_start(out=wb[:, :], in_=w_t[:, :].to_broadcast([Q, 6])).then_inc(
            dW, 16
        )

        # --- vector: sk[p,n] = sum_d key[p,n,d] * wk[d]  (wk = W[0, 3:6]):
        # one multiply against a 0-step-middle broadcast of wk, one reduce ---
        nc.vector.memset(bz[:, :], 0.0)
        nc.vector.memset(ones[:, :], 1.0)
        nc.vector.wait_ge(dK, 16)
        nc.vector.wait_ge(dW, 16)
        wk_b = bass.AP(tensor=wb.ap().tensor, offset=3, ap=[[6, Q], [0, QF], [1, D]])
        nc.vector.tensor_mul(prod[:, :, :], kv[:, :, :], wk_b).then_inc(vp)
        nc.vector.wait_ge(vp, 1)
        nc.vector.reduce_sum(
            sk[:, :], prod[:, :, :], axis=mybir.AxisListType.X
        ).then_inc(s_sk, 1)

        # --- scalar: e = exp(sk), written into mall4 row 3 ---
        nc.scalar.wait_ge(s_sk, 1)
        nc.scalar.activation(
            mall4[:, 3, :], sk[:, :], mybir.ActivationFunctionType.Exp,
            bias=bz[:, 0:1],
        ).then_inc(s_e, 1)

        # --- vector: mall4 rows 0-2 = e * value_d (d-major strided write),
        # then ONE reduce -> pcat[:, 0:4] = [num0, num1, num2, Z] partials ---
        nc.vector.wait_ge(s_e, 1)
        nc.vector.wait_ge(dV, 16)
        e_b = bass.AP(
            tensor=mall4.ap().tensor, offset=3 * QF,
            ap=[[4 * QF, Q], [1, QF], [0, D]],
        )
        m_out = bass.AP(
            tensor=mall4.ap().tensor, offset=0,
            ap=[[4 * QF, Q], [1, QF], [QF, D]],
        )
        nc.vector.tensor_mul(m_out, e_b, vv[:, :, :]).then_inc(vp)
        nc.vector.wait_ge(vp, 2)
        nc.vector.reduce_sum(
            pcat[:, :], mall4[:, :, :], axis=mybir.AxisListType.X
        ).then_inc(s_pc, 1)

        # --- tensor: ones(32,32).T @ pcat(32,4) reduces across partitions
        # AND broadcasts the 4 column sums to all 32 partitions in one op ---
        nc.tensor.wait_ge(s_pc, 1)
        nc.tensor.matmul(
            redp.ap()[:, :], ones[:, :], pcat[:, :], start=True, stop=True
        ).then_inc(s_mm, 1)

        # --- vector: v* = num/Z straight out of PSUM, broadcast x64 into the
        # out tile via a 0-step middle dim on the input AP ---
        nc.vector.wait_ge(s_mm, 1)
        nc.vector.reciprocal(rz[:, :], redp.ap()[:, 3:4]).then_inc(vp)
        nc.vector.wait_ge(vp, 3)
        num_b = bass.AP(
            tensor=redp.ap().tensor, offset=0, ap=[[4, Q], [0, QF], [1, D]]
        )
        nc.vector.tensor_scalar_mul(ot[:, :, :], num_b, rz[:, 0:1]).then_inc(s_ot, 1)

        # --- scalar: store the full shard. No explicit dO wait: the
        # runtime-injected epilogue DRAIN on ACT covers the in-flight DMA,
        # so its round trip overlaps the (fixed) exit sweep ---
        nc.scalar.wait_ge(s_ot, 1)
        nc.scalar.dma_start(
            out=out_t[:, :].rearrange("(p n) d -> p n d", p=Q), in_=ot[:, :, :]
        ).then_inc(dO, 16)

    _strip_framework_overhead(nc)
    nc.compile()
    return nc


def kernel(x, key, value, W, b):
    global last_result, _nc_cache
    key = np.ascontiguousarray(np.asarray(key, dtype=np.float32))
    value = np.ascontiguousarray(np.asarray(value, dtype=np.float32))
    W = np.ascontiguousarray(np.asarray(W, dtype=np.float32))
    if _nc_cache is None:
        _nc_cache = _build()
    in_maps = [
        {"key": key, "value": value, "W": W} for _ in range(N_CORES)
    ]
    res = run_bass_kernel_spmd(_nc_cache, in_maps, core_ids=list(range(N_CORES)))
    last_result = res
    out = np.concatenate([r["out"] for r in res.results], axis=0)
    return out.reshape(B, S1, D)
